# revision 10
# baseline (speedup 1.0000x reference)
"""Multi-head attention kernel for 8 Trainium2 NeuronCores.

Problem: B=4, S=2048, D=1024, H=16 heads (d_k=64), fp32 inputs,
random 0/1 attention mask [B, S, S].

Sharding: core c -> (batch b = c//2, head-group g = c%2).  Each core
computes 8 heads of one batch: Megatron column-parallel QKV, row-parallel
output projection.  Host sums the two partial outputs per batch.

Device-side layout choices (avoids every on-device transpose):
  - host passes x^T [D, S] so projections contract D on partitions
  - projections emit qh^T / kh^T [512, S] (head dims on partitions)
  - scores are computed transposed: S^T[k, q] = kh^T.T @ qh^T
  - softmax: exp on ScalarE (no max subtraction; scores are O(5)),
    multiplicative fp16 {0,1} mask on VectorE (2x packed mode),
    denominator = ones-column appended to V in the P@V matmul
  - ctx^T[d, q] accumulates in PSUM; normalization multiplies by a
    reciprocal row broadcast across partitions via a tiny K=2
    block-diagonal matmul
  - output projection consumes ctx^T directly, emits out^T partials
"""

import numpy as np

B = 4
S = 2048
D = 1024
H = 16  # total heads
HL = 8  # heads per core
DK = 64
DH = HL * DK  # 512 local head dims
P = 128
N_CORES = 8

_compiled = None


def _build_program():
    import concourse.bacc as bacc
    import concourse.tile as tile
    from concourse import mybir

    f32 = mybir.dt.float32
    f32r = mybir.dt.float32r
    f16 = mybir.dt.float16
    AF = mybir.ActivationFunctionType

    nc = bacc.Bacc()

    # ---- DRAM I/O ----
    xqT = nc.declare_dram_parameter("xqT", [D, S], f32r, isOutput=False)
    xkT = nc.declare_dram_parameter("xkT", [D, S], f32r, isOutput=False)
    xvT = nc.declare_dram_parameter("xvT", [D, S], f32r, isOutput=False)
    maskT = nc.declare_dram_parameter("maskT", [S, S], f16, isOutput=False)
    wqT = nc.declare_dram_parameter("wqT", [D, DH], f32r, isOutput=False)
    wkT = nc.declare_dram_parameter("wkT", [D, DH], f32r, isOutput=False)
    wvT = nc.declare_dram_parameter("wvT", [D, DH], f32r, isOutput=False)
    woT = nc.declare_dram_parameter("woT", [DH, D], f16, isOutput=False)
    bq = nc.declare_dram_parameter("bq", [DH], f32, isOutput=False)
    bk = nc.declare_dram_parameter("bk", [DH], f32, isOutput=False)
    bv = nc.declare_dram_parameter("bv", [DH], f32, isOutput=False)
    bo = nc.declare_dram_parameter("bo", [D], f32, isOutput=False)
    outT = nc.declare_dram_parameter("outT", [D, S], f32, isOutput=True)

    KC = D // P       # 8 contraction chunks for QKV projections
    DT = DH // P      # 4 dim-tiles of qh^T/kh^T
    SC = S // 512     # 4 seq chunks of 512
    ST = S // P       # 16 seq tiles of 128
    OT = D // P       # 8 output dim tiles
    CC = DH // P      # 4 contraction chunks for O-projection
    PAIRS = HL // 2   # 4 head pairs

    with tile.TileContext(nc) as tc:
        with tc.tile_pool(name="persist", bufs=1) as persist:
            qhT_sb = persist.tile([P, DT, S], f16)
            khT_sb = persist.tile([P, DT, S], f16)
            vh_sb = persist.tile([P, ST, HL * (DK + 1)], f16)
            ctxT_sb = persist.tile([P, CC, S], f16)
            wo_sb = persist.tile([P, CC, D], f16)
            bq_sb = persist.tile([P, DT], f32)
            bk_sb = persist.tile([P, DT], f32)
            bo_sb = persist.tile([P, OT], f32)
            bv_bc = persist.tile([P, DH], f32)

            # constants / small loads
            nc.sync.dma_start(out=bq_sb, in_=bq[:].rearrange("(t p) -> p t", p=P))
            nc.sync.dma_start(out=bk_sb, in_=bk[:].rearrange("(t p) -> p t", p=P))
            nc.sync.dma_start(out=bo_sb, in_=bo[:].rearrange("(t p) -> p t", p=P))
            nc.sync.dma_start(out=bv_bc, in_=bv[:].unsqueeze(0).to_broadcast((P, DH)))
            # ones columns of vh (for softmax denominators)
            nc.vector.memset(
                vh_sb.rearrange("p t (h c) -> p t h c", c=DK + 1)[:, :, :, DK : DK + 1],
                1.0,
            )

            # ================= Phase 1: QKV projections =================
            with (
                tc.tile_pool(name="wpool", bufs=1) as wpool,
                tc.tile_pool(name="xs", bufs=3) as xs,
                tc.tile_pool(name="ps1", bufs=8, space="PSUM") as ps1,
            ):
                wq_sb = wpool.tile([P, KC, DH], f32r)
                wk_sb = wpool.tile([P, KC, DH], f32r)
                wv_sb = wpool.tile([P, KC, DH], f32r)
                nc.sync.dma_start(out=wq_sb, in_=wqT[:, :].rearrange("(c p) m -> p c m", p=P))
                nc.sync.dma_start(out=wk_sb, in_=wkT[:, :].rearrange("(c p) m -> p c m", p=P))
                nc.sync.dma_start(out=wv_sb, in_=wvT[:, :].rearrange("(c p) m -> p c m", p=P))
                nc.sync.dma_start(out=wo_sb, in_=woT[:, :].rearrange("(c p) m -> p c m", p=P))

                # qh^T and kh^T: [dim-tile 128, seq 512] tiles
                for sc in range(SC):
                    sl = slice(sc * 512, (sc + 1) * 512)
                    psq = [ps1.tile([P, 512], f32, name=f"psq{sc}_{i}", tag="ps1t") for i in range(DT)]
                    psk = [ps1.tile([P, 512], f32, name=f"psk{sc}_{i}", tag="ps1t") for i in range(DT)]
                    for kc in range(KC):
                        xq_t = xs.tile([P, 512], f32r, name="xq_t")
                        xk_t = xs.tile([P, 512], f32r, name="xk_t")
                        nc.sync.dma_start(out=xq_t, in_=xqT[kc * P : (kc + 1) * P, sl])
                        nc.sync.dma_start(out=xk_t, in_=xkT[kc * P : (kc + 1) * P, sl])
                        for dt_ in range(DT):
                            wslice = slice(dt_ * P, (dt_ + 1) * P)
                            nc.tensor.matmul(
                                psq[dt_][:, :],
                                lhsT=wq_sb[:, kc, wslice],
                                rhs=xq_t[:, :],
                                start=(kc == 0),
                                stop=(kc == KC - 1),
                            )
                            nc.tensor.matmul(
                                psk[dt_][:, :],
                                lhsT=wk_sb[:, kc, wslice],
                                rhs=xk_t[:, :],
                                start=(kc == 0),
                                stop=(kc == KC - 1),
                            )
                    for dt_ in range(DT):
                        nc.vector.tensor_scalar_add(
                            out=qhT_sb[:, dt_, sl],
                            in0=psq[dt_][:, :],
                            scalar1=bq_sb[:, dt_ : dt_ + 1],
                        )
                        nc.vector.tensor_scalar_add(
                            out=khT_sb[:, dt_, sl],
                            in0=psk[dt_][:, :],
                            scalar1=bk_sb[:, dt_ : dt_ + 1],
                        )

                # vh: [seq-tile 128, 512 head dims] tiles, strided per head
                vh_heads = vh_sb.rearrange("p t (h c) -> p t h c", c=DK + 1)
                bv_heads = bv_bc.rearrange("p (h c) -> p h c", c=DK)
                for st in range(ST):
                    xv_t = xs.tile([P, KC, P], f32r, name="xv_t")
                    nc.sync.dma_start(
                        out=xv_t,
                        in_=xvT[:, st * P : (st + 1) * P].rearrange(
                            "(c p) j -> p c j", p=P
                        ),
                    )
                    psv = ps1.tile([P, 512], f32, name="psv", tag="ps1t")
                    for kc in range(KC):
                        nc.tensor.matmul(
                            psv[:, :],
                            lhsT=xv_t[:, kc, :],
                            rhs=wv_sb[:, kc, :],
                            start=(kc == 0),
                            stop=(kc == KC - 1),
                        )
                    nc.vector.tensor_add(
                        vh_heads[:, st, :, 0:DK],
                        psv[:, :].rearrange("p (h c) -> p h c", c=DK),
                        bv_heads[:, :, :],
                    )

            # ================= Phase 2: attention =================
            with (
                tc.tile_pool(name="maskp", bufs=2) as maskp,
                tc.tile_pool(name="pt", bufs=4) as ptp,
                tc.tile_pool(name="small", bufs=4) as small,
                tc.tile_pool(name="ps2", bufs=8, space="PSUM") as ps2,
            ):
                for qc in range(SC):
                    qsl = slice(qc * 512, (qc + 1) * 512)
                    m_sb = maskp.tile([P, ST, 512], f16, name="m_sb")
                    nc.sync.dma_start(
                        out=m_sb,
                        in_=maskT[:, qsl].rearrange("(t p) j -> p t j", p=P),
                    )
                    for pair in range(PAIRS):
                        hA, hB = 2 * pair, 2 * pair + 1
                        ctx_A = ps2.tile([DK + 1, 512], f32, name="ctx_A", tag="ctxps", bufs=2)
                        ctx_B = ps2.tile([DK + 1, 512], f32, name="ctx_B", tag="ctxps", bufs=2)
                        for kt in range(ST):
                            ksl = slice(kt * P, (kt + 1) * P)
                            s_A = ps2.tile([P, 512], f32, name="s_A", tag="sps", bufs=4)
                            s_B = ps2.tile([P, 512], f32, name="s_B", tag="sps", bufs=4)
                            nc.tensor.matmul(
                                s_A[:, :],
                                lhsT=khT_sb[0:DK, pair, ksl],
                                rhs=qhT_sb[0:DK, pair, qsl],
                                tile_position=(0, 0),
                            )
                            nc.tensor.matmul(
                                s_B[:, :],
                                lhsT=khT_sb[DK : 2 * DK, pair, ksl],
                                rhs=qhT_sb[DK : 2 * DK, pair, qsl],
                                tile_position=(DK, 0),
                            )
                            p_A = ptp.tile([P, 512], f16, name="p_A")
                            p_B = ptp.tile([P, 512], f16, name="p_B")
                            nc.scalar.activation(p_A[:, :], s_A[:, :], AF.Exp)
                            nc.scalar.activation(p_B[:, :], s_B[:, :], AF.Exp)
                            nc.vector.tensor_mul(p_A[:, :], p_A[:, :], m_sb[:, kt, :])
                            nc.vector.tensor_mul(p_B[:, :], p_B[:, :], m_sb[:, kt, :])
                            nc.tensor.matmul(
                                ctx_A[:, :],
                                lhsT=vh_sb[:, kt, hA * (DK + 1) : (hA + 1) * (DK + 1)],
                                rhs=p_A[:, :],
                                start=(kt == 0),
                                stop=(kt == ST - 1),
                            )
                            nc.tensor.matmul(
                                ctx_B[:, :],
                                lhsT=vh_sb[:, kt, hB * (DK + 1) : (hB + 1) * (DK + 1)],
                                rhs=p_B[:, :],
                                start=(kt == 0),
                                stop=(kt == ST - 1),
                            )
                        # normalization: recip of denominator rows, broadcast
                        # across partitions on GpSimd, multiply on VectorE.
                        # Head B's 64 rows then move to partitions 64-127 of
                        # ctxT via a small SBUF->SBUF DMA (engines cannot
                        # shift data across partitions).
                        recips = small.tile([1, 2, 512], f16, name="recips")
                        with nc.allow_low_precision(reason="softmax recip in f16"):
                            nc.vector.reciprocal(recips[0:1, 0, :], ctx_A[DK : DK + 1, :])
                            nc.vector.reciprocal(recips[0:1, 1, :], ctx_B[DK : DK + 1, :])
                        bcA = small.tile([DK, 512], f16, name="bcA")
                        bcB = small.tile([DK, 512], f16, name="bcB")
                        nc.gpsimd.partition_broadcast(bcA[:, :], recips[0:1, 0, :])
                        nc.gpsimd.partition_broadcast(bcB[:, :], recips[0:1, 1, :])
                        nc.vector.tensor_mul(
                            ctxT_sb[0:DK, pair, qsl], ctx_A[0:DK, :], bcA[:, :]
                        )
                        stgB = small.tile([DK, 512], f16, name="stgB")
                        nc.vector.tensor_mul(stgB[:, :], ctx_B[0:DK, :], bcB[:, :])
                        nc.sync.dma_start(
                            out=ctxT_sb[DK : 2 * DK, pair, qsl], in_=stgB[:, :]
                        )

            # ================= Phase 3: output projection =================
            with (
                tc.tile_pool(name="outp", bufs=3) as outp,
                tc.tile_pool(name="ps3", bufs=4, space="PSUM") as ps3,
            ):
                for ot in range(OT):
                    for sc in range(SC):
                        sl = slice(sc * 512, (sc + 1) * 512)
                        pso = ps3.tile([P, 512], f32, name="pso")
                        for cc in range(CC):
                            nc.tensor.matmul(
                                pso[:, :],
                                lhsT=wo_sb[:, cc, ot * P : (ot + 1) * P],
                                rhs=ctxT_sb[:, cc, sl],
                                start=(cc == 0),
                                stop=(cc == CC - 1),
                            )
                        o_sb = outp.tile([P, 512], f32, name="o_sb")
                        nc.vector.tensor_scalar_add(
                            out=o_sb[:, :], in0=pso[:, :], scalar1=bo_sb[:, ot : ot + 1]
                        )
                        nc.sync.dma_start(
                            out=outT[ot * P : (ot + 1) * P, sl], in_=o_sb[:, :]
                        )

    nc.finalize()
    return nc


def _shard_inputs(q, k, v, mask, Wq, bq, Wk, bk, Wv, bv, Wo, bo):
    """Build the 8 per-core input maps."""
    scale = np.float32(1.0 / np.sqrt(DK))
    in_maps = []
    per_batch = {}
    for b in range(B):
        per_batch[b] = dict(
            xqT=np.ascontiguousarray(q[b].T, dtype=np.float32),
            xkT=np.ascontiguousarray(k[b].T, dtype=np.float32),
            xvT=np.ascontiguousarray(v[b].T, dtype=np.float32),
            maskT=np.ascontiguousarray(mask[b].T).astype(np.float16),
        )
    zeros_bo = np.zeros_like(bo, dtype=np.float32)
    for c in range(N_CORES):
        b, g = c // 2, c % 2
        hsl = slice(g * DH, (g + 1) * DH)
        in_maps.append(
            dict(
                per_batch[b],
                wqT=np.ascontiguousarray((Wq[hsl, :] * scale).T, dtype=np.float32),
                wkT=np.ascontiguousarray(Wk[hsl, :].T, dtype=np.float32),
                wvT=np.ascontiguousarray(Wv[hsl, :].T, dtype=np.float32),
                woT=np.ascontiguousarray(Wo[:, hsl].T).astype(np.float16),
                bq=np.ascontiguousarray(bq[hsl] * scale, dtype=np.float32),
                bk=np.ascontiguousarray(bk[hsl], dtype=np.float32),
                bv=np.ascontiguousarray(bv[hsl], dtype=np.float32),
                bo=(bo.astype(np.float32) if g == 0 else zeros_bo),
            )
        )
    return in_maps


def run_on_cores(in_maps, trace=False):
    global _compiled
    from concourse import bass_utils

    if _compiled is None:
        _compiled = _build_program()
    res = bass_utils.run_bass_kernel_spmd(
        _compiled, in_maps, core_ids=list(range(N_CORES)), trace=trace
    )
    return res


def kernel(q, k, v, mask, Wq, bq, Wk, bk, Wv, bv, Wo, bo):
    in_maps = _shard_inputs(q, k, v, mask, Wq, bq, Wk, bk, Wv, bv, Wo, bo)
    res = run_on_cores(in_maps)
    out = np.empty((B, S, D), dtype=np.float32)
    for b in range(B):
        partial = res.results[2 * b]["outT"] + res.results[2 * b + 1]["outT"]
        out[b] = partial.T
    return out


# revision 13
# speedup vs baseline: 1.0973x; 1.0973x over previous
"""Multi-head attention kernel for 8 Trainium2 NeuronCores.

Problem: B=4, S=2048, D=1024, H=16 heads (d_k=64), fp32 inputs,
random 0/1 attention mask [B, S, S].

Sharding: core c -> (batch b = c//2, head-group g = c%2).  Each core
computes 8 heads of one batch: Megatron column-parallel QKV, row-parallel
output projection.  Host sums the two partial outputs per batch.

Device-side layout choices (avoids every on-device transpose):
  - host passes x^T [D, S] so projections contract D on partitions
  - projections emit qh^T / kh^T [512, S] (head dims on partitions)
  - scores are computed transposed: S^T[k, q] = kh^T.T @ qh^T
  - softmax: exp on ScalarE (no max subtraction; scores are O(5)),
    multiplicative fp16 {0,1} mask on VectorE (2x packed mode),
    denominator = ones-column appended to V in the P@V matmul
  - ctx^T[d, q] accumulates in PSUM; normalization multiplies by a
    reciprocal row broadcast across partitions via a tiny K=2
    block-diagonal matmul
  - output projection consumes ctx^T directly, emits out^T partials
"""

import numpy as np

B = 4
S = 2048
D = 1024
H = 16  # total heads
HL = 8  # heads per core
DK = 64
DH = HL * DK  # 512 local head dims
P = 128
N_CORES = 8

_compiled = None


def _build_program():
    import concourse.bacc as bacc
    import concourse.tile as tile
    from concourse import mybir

    f32 = mybir.dt.float32
    f32r = mybir.dt.float32r
    f16 = mybir.dt.float16
    AF = mybir.ActivationFunctionType

    nc = bacc.Bacc()

    # ---- DRAM I/O ----
    xqT = nc.declare_dram_parameter("xqT", [D, S], f32r, isOutput=False)
    xkT = nc.declare_dram_parameter("xkT", [D, S], f32r, isOutput=False)
    xvT = nc.declare_dram_parameter("xvT", [D, S], f32r, isOutput=False)
    maskT = nc.declare_dram_parameter("maskT", [S, S], f16, isOutput=False)
    wqT = nc.declare_dram_parameter("wqT", [D, DH], f32r, isOutput=False)
    wkT = nc.declare_dram_parameter("wkT", [D, DH], f32r, isOutput=False)
    wvT = nc.declare_dram_parameter("wvT", [D, DH], f32r, isOutput=False)
    woT = nc.declare_dram_parameter("woT", [DH, D], f16, isOutput=False)
    bq = nc.declare_dram_parameter("bq", [DH], f32, isOutput=False)
    bk = nc.declare_dram_parameter("bk", [DH], f32, isOutput=False)
    bv = nc.declare_dram_parameter("bv", [DH], f32, isOutput=False)
    bo = nc.declare_dram_parameter("bo", [D], f32, isOutput=False)
    outT = nc.declare_dram_parameter("outT", [D, S], f32, isOutput=True)

    KC = D // P       # 8 contraction chunks for QKV projections
    DT = DH // P      # 4 dim-tiles of qh^T/kh^T
    SC = S // 512     # 4 seq chunks of 512
    ST = S // P       # 16 seq tiles of 128
    OT = D // P       # 8 output dim tiles
    CC = DH // P      # 4 contraction chunks for O-projection
    PAIRS = HL // 2   # 4 head pairs

    with tile.TileContext(nc) as tc:
        with tc.tile_pool(name="persist", bufs=1) as persist:
            qhT_sb = persist.tile([P, DT, S], f16)
            khT_sb = persist.tile([P, DT, S], f16)
            vh_sb = persist.tile([P, ST, HL * (DK + 1)], f16)
            ctxT_sb = persist.tile([P, CC, S], f16)
            wo_sb = persist.tile([P, CC, D], f16)
            bq_sb = persist.tile([P, DT], f32)
            bk_sb = persist.tile([P, DT], f32)
            bo_sb = persist.tile([P, OT], f32)
            bv_bc = persist.tile([P, DH], f32)

            # constants / small loads
            nc.sync.dma_start(out=bq_sb, in_=bq[:].rearrange("(t p) -> p t", p=P))
            nc.sync.dma_start(out=bk_sb, in_=bk[:].rearrange("(t p) -> p t", p=P))
            nc.sync.dma_start(out=bo_sb, in_=bo[:].rearrange("(t p) -> p t", p=P))
            nc.sync.dma_start(out=bv_bc, in_=bv[:].unsqueeze(0).to_broadcast((P, DH)))
            # ones columns of vh (for softmax denominators)
            nc.vector.memset(
                vh_sb.rearrange("p t (h c) -> p t h c", c=DK + 1)[:, :, :, DK : DK + 1],
                1.0,
            )

            # ================= Phase 1: QKV projections =================
            with (
                tc.tile_pool(name="wpool", bufs=1) as wpool,
                tc.tile_pool(name="xs", bufs=3) as xs,
                tc.tile_pool(name="ps1", bufs=8, space="PSUM") as ps1,
            ):
                wq_sb = wpool.tile([P, KC, DH], f32r)
                wk_sb = wpool.tile([P, KC, DH], f32r)
                wv_sb = wpool.tile([P, KC, DH], f32r)
                nc.sync.dma_start(out=wq_sb, in_=wqT[:, :].rearrange("(c p) m -> p c m", p=P))
                nc.sync.dma_start(out=wk_sb, in_=wkT[:, :].rearrange("(c p) m -> p c m", p=P))
                nc.sync.dma_start(out=wv_sb, in_=wvT[:, :].rearrange("(c p) m -> p c m", p=P))
                nc.sync.dma_start(out=wo_sb, in_=woT[:, :].rearrange("(c p) m -> p c m", p=P))

                # qh^T and kh^T: [dim-tile 128, seq 512] tiles
                for sc in range(SC):
                    sl = slice(sc * 512, (sc + 1) * 512)
                    psq = [ps1.tile([P, 512], f32, name=f"psq{sc}_{i}", tag="ps1t") for i in range(DT)]
                    psk = [ps1.tile([P, 512], f32, name=f"psk{sc}_{i}", tag="ps1t") for i in range(DT)]
                    for kc in range(KC):
                        xq_t = xs.tile([P, 512], f32r, name="xq_t")
                        xk_t = xs.tile([P, 512], f32r, name="xk_t")
                        nc.sync.dma_start(out=xq_t, in_=xqT[kc * P : (kc + 1) * P, sl])
                        nc.sync.dma_start(out=xk_t, in_=xkT[kc * P : (kc + 1) * P, sl])
                        for dt_ in range(DT):
                            wslice = slice(dt_ * P, (dt_ + 1) * P)
                            nc.tensor.matmul(
                                psq[dt_][:, :],
                                lhsT=wq_sb[:, kc, wslice],
                                rhs=xq_t[:, :],
                                start=(kc == 0),
                                stop=(kc == KC - 1),
                            )
                            nc.tensor.matmul(
                                psk[dt_][:, :],
                                lhsT=wk_sb[:, kc, wslice],
                                rhs=xk_t[:, :],
                                start=(kc == 0),
                                stop=(kc == KC - 1),
                            )
                    for dt_ in range(DT):
                        nc.vector.tensor_scalar_add(
                            out=qhT_sb[:, dt_, sl],
                            in0=psq[dt_][:, :],
                            scalar1=bq_sb[:, dt_ : dt_ + 1],
                        )
                        nc.vector.tensor_scalar_add(
                            out=khT_sb[:, dt_, sl],
                            in0=psk[dt_][:, :],
                            scalar1=bk_sb[:, dt_ : dt_ + 1],
                        )

                # vh: [seq-tile 128, 512 head dims] tiles, strided per head
                vh_heads = vh_sb.rearrange("p t (h c) -> p t h c", c=DK + 1)
                bv_heads = bv_bc.rearrange("p (h c) -> p h c", c=DK)
                for st in range(ST):
                    xv_t = xs.tile([P, KC, P], f32r, name="xv_t")
                    nc.sync.dma_start(
                        out=xv_t,
                        in_=xvT[:, st * P : (st + 1) * P].rearrange(
                            "(c p) j -> p c j", p=P
                        ),
                    )
                    psv = ps1.tile([P, 512], f32, name="psv", tag="ps1t")
                    for kc in range(KC):
                        nc.tensor.matmul(
                            psv[:, :],
                            lhsT=xv_t[:, kc, :],
                            rhs=wv_sb[:, kc, :],
                            start=(kc == 0),
                            stop=(kc == KC - 1),
                        )
                    nc.vector.tensor_add(
                        vh_heads[:, st, :, 0:DK],
                        psv[:, :].rearrange("p (h c) -> p h c", c=DK),
                        bv_heads[:, :, :],
                    )

            # ================= Phase 2: attention =================
            with (
                tc.tile_pool(name="maskp", bufs=2) as maskp,
                tc.tile_pool(name="pt", bufs=6) as ptp,
                tc.tile_pool(name="small", bufs=4) as small,
                tc.tile_pool(name="ps2", bufs=8, space="PSUM") as ps2,
            ):
                for qc in range(SC):
                    qsl = slice(qc * 512, (qc + 1) * 512)
                    m_sb = maskp.tile([P, ST, 512], f16, name="m_sb")
                    nc.sync.dma_start(
                        out=m_sb,
                        in_=maskT[:, qsl].rearrange("(t p) j -> p t j", p=P),
                    )
                    for pair in range(PAIRS):
                        hA, hB = 2 * pair, 2 * pair + 1
                        ctx_A = ps2.tile([DK + 1, 512], f32, name="ctx_A", tag="ctxps", bufs=4)
                        ctx_B = ps2.tile([DK + 1, 512], f32, name="ctx_B", tag="ctxps", bufs=4)
                        for kt in range(ST):
                            ksl = slice(kt * P, (kt + 1) * P)
                            s_A = ps2.tile([P, 512], f32, name="s_A", tag="sps", bufs=4)
                            s_B = ps2.tile([P, 512], f32, name="s_B", tag="sps", bufs=4)
                            nc.tensor.matmul(
                                s_A[:, :],
                                lhsT=khT_sb[0:DK, pair, ksl],
                                rhs=qhT_sb[0:DK, pair, qsl],
                                tile_position=(0, 0),
                            )
                            nc.tensor.matmul(
                                s_B[:, :],
                                lhsT=khT_sb[DK : 2 * DK, pair, ksl],
                                rhs=qhT_sb[DK : 2 * DK, pair, qsl],
                                tile_position=(DK, 0),
                            )
                            p_A = ptp.tile([P, 512], f16, name="p_A")
                            p_B = ptp.tile([P, 512], f16, name="p_B")
                            nc.scalar.activation(p_A[:, :], s_A[:, :], AF.Exp)
                            nc.scalar.activation(p_B[:, :], s_B[:, :], AF.Exp)
                            nc.vector.tensor_mul(p_A[:, :], p_A[:, :], m_sb[:, kt, :])
                            nc.vector.tensor_mul(p_B[:, :], p_B[:, :], m_sb[:, kt, :])
                            nc.tensor.matmul(
                                ctx_A[:, :],
                                lhsT=vh_sb[:, kt, hA * (DK + 1) : (hA + 1) * (DK + 1)],
                                rhs=p_A[:, :],
                                start=(kt == 0),
                                stop=(kt == ST - 1),
                            )
                            nc.tensor.matmul(
                                ctx_B[:, :],
                                lhsT=vh_sb[:, kt, hB * (DK + 1) : (hB + 1) * (DK + 1)],
                                rhs=p_B[:, :],
                                start=(kt == 0),
                                stop=(kt == ST - 1),
                            )
                        # normalization: recip of denominator rows, broadcast
                        # across partitions on GpSimd, multiply on VectorE.
                        # Head B's 64 rows then move to partitions 64-127 of
                        # ctxT via a small SBUF->SBUF DMA (engines cannot
                        # shift data across partitions).
                        dens = small.tile([1, 2, 512], f32, name="dens")
                        nc.vector.tensor_copy(dens[0:1, 0, :], ctx_A[DK : DK + 1, :])
                        nc.vector.tensor_copy(dens[0:1, 1, :], ctx_B[DK : DK + 1, :])
                        recips = small.tile([1, 2, 512], f32, name="recips")
                        nc.vector.reciprocal_approx_fast(
                            out=recips[0:1, :, :], in_=dens[0:1, :, :]
                        )
                        bcA = small.tile([DK, 512], f32, name="bcA")
                        bcB = small.tile([DK, 512], f32, name="bcB")
                        nc.gpsimd.partition_broadcast(bcA[:, :], recips[0:1, 0, :])
                        nc.gpsimd.partition_broadcast(bcB[:, :], recips[0:1, 1, :])
                        nc.vector.tensor_mul(
                            ctxT_sb[0:DK, pair, qsl], ctx_A[0:DK, :], bcA[:, :]
                        )
                        stgB = small.tile([DK, 512], f16, name="stgB")
                        nc.vector.tensor_mul(stgB[:, :], ctx_B[0:DK, :], bcB[:, :])
                        nc.sync.dma_start(
                            out=ctxT_sb[DK : 2 * DK, pair, qsl], in_=stgB[:, :]
                        )

            # ================= Phase 3: output projection =================
            with (
                tc.tile_pool(name="outp", bufs=3) as outp,
                tc.tile_pool(name="ps3", bufs=4, space="PSUM") as ps3,
            ):
                for ot in range(OT):
                    for sc in range(SC):
                        sl = slice(sc * 512, (sc + 1) * 512)
                        pso = ps3.tile([P, 512], f32, name="pso")
                        for cc in range(CC):
                            nc.tensor.matmul(
                                pso[:, :],
                                lhsT=wo_sb[:, cc, ot * P : (ot + 1) * P],
                                rhs=ctxT_sb[:, cc, sl],
                                start=(cc == 0),
                                stop=(cc == CC - 1),
                            )
                        o_sb = outp.tile([P, 512], f32, name="o_sb")
                        nc.vector.tensor_scalar_add(
                            out=o_sb[:, :], in0=pso[:, :], scalar1=bo_sb[:, ot : ot + 1]
                        )
                        nc.sync.dma_start(
                            out=outT[ot * P : (ot + 1) * P, sl], in_=o_sb[:, :]
                        )

    nc.finalize()
    return nc


def _shard_inputs(q, k, v, mask, Wq, bq, Wk, bk, Wv, bv, Wo, bo):
    """Build the 8 per-core input maps."""
    scale = np.float32(1.0 / np.sqrt(DK))
    in_maps = []
    per_batch = {}
    for b in range(B):
        per_batch[b] = dict(
            xqT=np.ascontiguousarray(q[b].T, dtype=np.float32),
            xkT=np.ascontiguousarray(k[b].T, dtype=np.float32),
            xvT=np.ascontiguousarray(v[b].T, dtype=np.float32),
            maskT=np.ascontiguousarray(mask[b].T).astype(np.float16),
        )
    zeros_bo = np.zeros_like(bo, dtype=np.float32)
    for c in range(N_CORES):
        b, g = c // 2, c % 2
        hsl = slice(g * DH, (g + 1) * DH)
        in_maps.append(
            dict(
                per_batch[b],
                wqT=np.ascontiguousarray((Wq[hsl, :] * scale).T, dtype=np.float32),
                wkT=np.ascontiguousarray(Wk[hsl, :].T, dtype=np.float32),
                wvT=np.ascontiguousarray(Wv[hsl, :].T, dtype=np.float32),
                woT=np.ascontiguousarray(Wo[:, hsl].T).astype(np.float16),
                bq=np.ascontiguousarray(bq[hsl] * scale, dtype=np.float32),
                bk=np.ascontiguousarray(bk[hsl], dtype=np.float32),
                bv=np.ascontiguousarray(bv[hsl], dtype=np.float32),
                bo=(bo.astype(np.float32) if g == 0 else zeros_bo),
            )
        )
    return in_maps


def run_on_cores(in_maps, trace=False):
    global _compiled
    from concourse import bass_utils

    if _compiled is None:
        _compiled = _build_program()
    res = bass_utils.run_bass_kernel_spmd(
        _compiled, in_maps, core_ids=list(range(N_CORES)), trace=trace
    )
    return res


def kernel(q, k, v, mask, Wq, bq, Wk, bk, Wv, bv, Wo, bo):
    in_maps = _shard_inputs(q, k, v, mask, Wq, bq, Wk, bk, Wv, bv, Wo, bo)
    res = run_on_cores(in_maps)
    out = np.empty((B, S, D), dtype=np.float32)
    for b in range(B):
        partial = res.results[2 * b]["outT"] + res.results[2 * b + 1]["outT"]
        out[b] = partial.T
    return out


# revision 15
# speedup vs baseline: 1.5923x; 1.4512x over previous
"""Multi-head attention kernel for 8 Trainium2 NeuronCores.

Problem: B=4, S=2048, D=1024, H=16 heads (d_k=64), fp32 inputs,
random 0/1 attention mask [B, S, S].

Sharding: core c -> (batch b = c//2, head-group g = c%2).  Each core
computes 8 heads of one batch: Megatron column-parallel QKV, row-parallel
output projection.  Host sums the two partial outputs per batch.

Device-side layout choices (avoids every on-device transpose):
  - host passes x^T [D, S] so projections contract D on partitions
  - projections emit qh^T / kh^T [512, S] (head dims on partitions)
  - scores are computed transposed: S^T[k, q] = kh^T.T @ qh^T
  - softmax: exp on ScalarE (no max subtraction; scores are O(5)),
    multiplicative fp16 {0,1} mask on VectorE (2x packed mode),
    denominator = ones-column appended to V in the P@V matmul
  - ctx^T[d, q] accumulates in PSUM; normalization multiplies by a
    reciprocal row broadcast across partitions via a tiny K=2
    block-diagonal matmul
  - output projection consumes ctx^T directly, emits out^T partials
"""

import numpy as np

B = 4
S = 2048
D = 1024
H = 16  # total heads
HL = 8  # heads per core
DK = 64
DH = HL * DK  # 512 local head dims
P = 128
N_CORES = 8

_compiled = None


def _build_program():
    import concourse.bacc as bacc
    import concourse.tile as tile
    from concourse import mybir

    f32 = mybir.dt.float32
    f32r = mybir.dt.float32r
    f16 = mybir.dt.float16
    AF = mybir.ActivationFunctionType

    nc = bacc.Bacc()

    # ---- DRAM I/O ----
    xqT = nc.declare_dram_parameter("xqT", [D, S], f32r, isOutput=False)
    xkT = nc.declare_dram_parameter("xkT", [D, S], f32r, isOutput=False)
    xvT = nc.declare_dram_parameter("xvT", [D, S], f32r, isOutput=False)
    maskT = nc.declare_dram_parameter("maskT", [S, S], f16, isOutput=False)
    wqT = nc.declare_dram_parameter("wqT", [D, DH], f32r, isOutput=False)
    wkT = nc.declare_dram_parameter("wkT", [D, DH], f32r, isOutput=False)
    wvT = nc.declare_dram_parameter("wvT", [D, DH], f32r, isOutput=False)
    woT = nc.declare_dram_parameter("woT", [DH, D], f16, isOutput=False)
    bq = nc.declare_dram_parameter("bq", [DH], f32, isOutput=False)
    bk = nc.declare_dram_parameter("bk", [DH], f32, isOutput=False)
    bv = nc.declare_dram_parameter("bv", [DH], f32, isOutput=False)
    bo = nc.declare_dram_parameter("bo", [D], f32, isOutput=False)
    outT = nc.declare_dram_parameter("outT", [D, S], f32, isOutput=True)

    KC = D // P       # 8 contraction chunks for QKV projections
    DT = DH // P      # 4 dim-tiles of qh^T/kh^T
    SC = S // 512     # 4 seq chunks of 512
    ST = S // P       # 16 seq tiles of 128
    OT = D // P       # 8 output dim tiles
    CC = DH // P      # 4 contraction chunks for O-projection
    PAIRS = HL // 2   # 4 head pairs

    with tile.TileContext(nc) as tc:
        with tc.tile_pool(name="persist", bufs=1) as persist:
            qhT_sb = persist.tile([P, DT, S], f16)
            khT_sb = persist.tile([P, PAIRS, 2, S], f16)
            vh_sb = persist.tile([P, ST, HL * P], f16)
            ctxT_sb = persist.tile([P, CC, S], f16)
            wo_sb = persist.tile([P, CC, D], f16)
            bq_sb = persist.tile([P, DT], f32)
            bk_sb = persist.tile([P, DT], f32)
            bo_sb = persist.tile([P, OT], f32)
            bv_bc = persist.tile([P, DH], f32)

            # constants / small loads
            nc.sync.dma_start(out=bq_sb, in_=bq[:].rearrange("(t p) -> p t", p=P))
            nc.sync.dma_start(out=bk_sb, in_=bk[:].rearrange("(t p) -> p t", p=P))
            nc.sync.dma_start(out=bo_sb, in_=bo[:].rearrange("(t p) -> p t", p=P))
            nc.sync.dma_start(out=bv_bc, in_=bv[:].unsqueeze(0).to_broadcast((P, DH)))
            # zero-init padded K/V layouts; ones columns of vh give
            # softmax denominators.  Zero weight rows/cols keep the PE
            # array fully engaged without changing results.
            nc.vector.memset(khT_sb[:, :, :, :], 0.0)
            nc.vector.memset(vh_sb[:, :, :], 0.0)
            nc.vector.memset(
                vh_sb.rearrange("p t (h c) -> p t h c", c=P)[:, :, :, DK : DK + 1],
                1.0,
            )

            # ================= Phase 1: QKV projections =================
            with (
                tc.tile_pool(name="wpool", bufs=1) as wpool,
                tc.tile_pool(name="xs", bufs=3) as xs,
                tc.tile_pool(name="ps1", bufs=8, space="PSUM") as ps1,
            ):
                wq_sb = wpool.tile([P, KC, DH], f32r)
                wk_sb = wpool.tile([P, KC, DH], f32r)
                wv_sb = wpool.tile([P, KC, DH], f32r)
                nc.sync.dma_start(out=wq_sb, in_=wqT[:, :].rearrange("(c p) m -> p c m", p=P))
                nc.sync.dma_start(out=wk_sb, in_=wkT[:, :].rearrange("(c p) m -> p c m", p=P))
                nc.sync.dma_start(out=wv_sb, in_=wvT[:, :].rearrange("(c p) m -> p c m", p=P))
                nc.sync.dma_start(out=wo_sb, in_=woT[:, :].rearrange("(c p) m -> p c m", p=P))

                # qh^T and kh^T: [dim-tile 128, seq 512] tiles
                for sc in range(SC):
                    sl = slice(sc * 512, (sc + 1) * 512)
                    psq = [ps1.tile([P, 512], f32, name=f"psq{sc}_{i}", tag="ps1t") for i in range(DT)]
                    psk = [ps1.tile([P, 512], f32, name=f"psk{sc}_{i}", tag="ps1t") for i in range(DT)]
                    for kc in range(KC):
                        xq_t = xs.tile([P, 512], f32r, name="xq_t")
                        xk_t = xs.tile([P, 512], f32r, name="xk_t")
                        nc.sync.dma_start(out=xq_t, in_=xqT[kc * P : (kc + 1) * P, sl])
                        nc.sync.dma_start(out=xk_t, in_=xkT[kc * P : (kc + 1) * P, sl])
                        for dt_ in range(DT):
                            wslice = slice(dt_ * P, (dt_ + 1) * P)
                            nc.tensor.matmul(
                                psq[dt_][:, :],
                                lhsT=wq_sb[:, kc, wslice],
                                rhs=xq_t[:, :],
                                start=(kc == 0),
                                stop=(kc == KC - 1),
                            )
                            nc.tensor.matmul(
                                psk[dt_][:, :],
                                lhsT=wk_sb[:, kc, wslice],
                                rhs=xk_t[:, :],
                                start=(kc == 0),
                                stop=(kc == KC - 1),
                            )
                    for dt_ in range(DT):
                        nc.vector.tensor_scalar_add(
                            out=qhT_sb[:, dt_, sl],
                            in0=psq[dt_][:, :],
                            scalar1=bq_sb[:, dt_ : dt_ + 1],
                        )
                        nc.vector.tensor_scalar_add(
                            out=khT_sb[0:DK, dt_, 0, sl],
                            in0=psk[dt_][0:DK, :],
                            scalar1=bk_sb[0:DK, dt_ : dt_ + 1],
                        )
                        nc.vector.tensor_scalar_add(
                            out=khT_sb[DK : 2 * DK, dt_, 1, sl],
                            in0=psk[dt_][DK : 2 * DK, :],
                            scalar1=bk_sb[DK : 2 * DK, dt_ : dt_ + 1],
                        )

                # vh: [seq-tile 128, 512 head dims] tiles, strided per head
                vh_heads = vh_sb.rearrange("p t (h c) -> p t h c", c=P)
                bv_heads = bv_bc.rearrange("p (h c) -> p h c", c=DK)
                for st in range(ST):
                    xv_t = xs.tile([P, KC, P], f32r, name="xv_t")
                    nc.sync.dma_start(
                        out=xv_t,
                        in_=xvT[:, st * P : (st + 1) * P].rearrange(
                            "(c p) j -> p c j", p=P
                        ),
                    )
                    psv = ps1.tile([P, 512], f32, name="psv", tag="ps1t")
                    for kc in range(KC):
                        nc.tensor.matmul(
                            psv[:, :],
                            lhsT=xv_t[:, kc, :],
                            rhs=wv_sb[:, kc, :],
                            start=(kc == 0),
                            stop=(kc == KC - 1),
                        )
                    nc.vector.tensor_add(
                        vh_heads[:, st, :, 0:DK],
                        psv[:, :].rearrange("p (h c) -> p h c", c=DK),
                        bv_heads[:, :, :],
                    )

            # ================= Phase 2: attention =================
            with (
                tc.tile_pool(name="maskp", bufs=2) as maskp,
                tc.tile_pool(name="pt", bufs=6) as ptp,
                tc.tile_pool(name="small", bufs=4) as small,
                tc.tile_pool(name="ps2", bufs=8, space="PSUM") as ps2,
            ):
                for qc in range(SC):
                    qsl = slice(qc * 512, (qc + 1) * 512)
                    m_sb = maskp.tile([P, ST, 512], f16, name="m_sb")
                    nc.sync.dma_start(
                        out=m_sb,
                        in_=maskT[:, qsl].rearrange("(t p) j -> p t j", p=P),
                    )
                    for pair in range(PAIRS):
                        hA, hB = 2 * pair, 2 * pair + 1
                        ctx_A = ps2.tile([P, 512], f32, name="ctx_A", tag="ctxps", bufs=4)
                        ctx_B = ps2.tile([P, 512], f32, name="ctx_B", tag="ctxps", bufs=4)
                        for kt in range(ST):
                            ksl = slice(kt * P, (kt + 1) * P)
                            s_A = ps2.tile([P, 512], f32, name="s_A", tag="sps", bufs=4)
                            s_B = ps2.tile([P, 512], f32, name="s_B", tag="sps", bufs=4)
                            nc.tensor.matmul(
                                s_A[:, :],
                                lhsT=khT_sb[:, pair, 0, ksl],
                                rhs=qhT_sb[:, pair, qsl],
                            )
                            nc.tensor.matmul(
                                s_B[:, :],
                                lhsT=khT_sb[:, pair, 1, ksl],
                                rhs=qhT_sb[:, pair, qsl],
                            )
                            p_A = ptp.tile([P, 512], f16, name="p_A")
                            p_B = ptp.tile([P, 512], f16, name="p_B")
                            nc.scalar.activation(p_A[:, :], s_A[:, :], AF.Exp)
                            nc.scalar.activation(p_B[:, :], s_B[:, :], AF.Exp)
                            nc.vector.tensor_mul(p_A[:, :], p_A[:, :], m_sb[:, kt, :])
                            nc.vector.tensor_mul(p_B[:, :], p_B[:, :], m_sb[:, kt, :])
                            nc.tensor.matmul(
                                ctx_A[:, :],
                                lhsT=vh_sb[:, kt, hA * P : (hA + 1) * P],
                                rhs=p_A[:, :],
                                start=(kt == 0),
                                stop=(kt == ST - 1),
                            )
                            nc.tensor.matmul(
                                ctx_B[:, :],
                                lhsT=vh_sb[:, kt, hB * P : (hB + 1) * P],
                                rhs=p_B[:, :],
                                start=(kt == 0),
                                stop=(kt == ST - 1),
                            )
                        # normalization: recip of denominator rows, broadcast
                        # across partitions on GpSimd, multiply on VectorE.
                        # Head B's 64 rows then move to partitions 64-127 of
                        # ctxT via a small SBUF->SBUF DMA (engines cannot
                        # shift data across partitions).
                        dens = small.tile([1, 2, 512], f32, name="dens")
                        nc.vector.tensor_copy(dens[0:1, 0, :], ctx_A[DK : DK + 1, :])
                        nc.vector.tensor_copy(dens[0:1, 1, :], ctx_B[DK : DK + 1, :])
                        recips = small.tile([1, 2, 512], f32, name="recips")
                        nc.vector.reciprocal_approx_fast(
                            out=recips[0:1, :, :], in_=dens[0:1, :, :]
                        )
                        bcA = small.tile([DK, 512], f32, name="bcA")
                        bcB = small.tile([DK, 512], f32, name="bcB")
                        nc.gpsimd.partition_broadcast(bcA[:, :], recips[0:1, 0, :])
                        nc.gpsimd.partition_broadcast(bcB[:, :], recips[0:1, 1, :])
                        nc.vector.tensor_mul(
                            ctxT_sb[0:DK, pair, qsl], ctx_A[0:DK, :], bcA[:, :]
                        )
                        stgB = small.tile([DK, 512], f16, name="stgB")
                        nc.vector.tensor_mul(stgB[:, :], ctx_B[0:DK, :], bcB[:, :])
                        nc.sync.dma_start(
                            out=ctxT_sb[DK : 2 * DK, pair, qsl], in_=stgB[:, :]
                        )

            # ================= Phase 3: output projection =================
            with (
                tc.tile_pool(name="outp", bufs=3) as outp,
                tc.tile_pool(name="ps3", bufs=4, space="PSUM") as ps3,
            ):
                for ot in range(OT):
                    for sc in range(SC):
                        sl = slice(sc * 512, (sc + 1) * 512)
                        pso = ps3.tile([P, 512], f32, name="pso")
                        for cc in range(CC):
                            nc.tensor.matmul(
                                pso[:, :],
                                lhsT=wo_sb[:, cc, ot * P : (ot + 1) * P],
                                rhs=ctxT_sb[:, cc, sl],
                                start=(cc == 0),
                                stop=(cc == CC - 1),
                            )
                        o_sb = outp.tile([P, 512], f32, name="o_sb")
                        nc.vector.tensor_scalar_add(
                            out=o_sb[:, :], in0=pso[:, :], scalar1=bo_sb[:, ot : ot + 1]
                        )
                        nc.sync.dma_start(
                            out=outT[ot * P : (ot + 1) * P, sl], in_=o_sb[:, :]
                        )

    nc.finalize()
    return nc


def _shard_inputs(q, k, v, mask, Wq, bq, Wk, bk, Wv, bv, Wo, bo):
    """Build the 8 per-core input maps."""
    scale = np.float32(1.0 / np.sqrt(DK))
    in_maps = []
    per_batch = {}
    for b in range(B):
        per_batch[b] = dict(
            xqT=np.ascontiguousarray(q[b].T, dtype=np.float32),
            xkT=np.ascontiguousarray(k[b].T, dtype=np.float32),
            xvT=np.ascontiguousarray(v[b].T, dtype=np.float32),
            maskT=np.ascontiguousarray(mask[b].T).astype(np.float16),
        )
    zeros_bo = np.zeros_like(bo, dtype=np.float32)
    for c in range(N_CORES):
        b, g = c // 2, c % 2
        hsl = slice(g * DH, (g + 1) * DH)
        in_maps.append(
            dict(
                per_batch[b],
                wqT=np.ascontiguousarray((Wq[hsl, :] * scale).T, dtype=np.float32),
                wkT=np.ascontiguousarray(Wk[hsl, :].T, dtype=np.float32),
                wvT=np.ascontiguousarray(Wv[hsl, :].T, dtype=np.float32),
                woT=np.ascontiguousarray(Wo[:, hsl].T).astype(np.float16),
                bq=np.ascontiguousarray(bq[hsl] * scale, dtype=np.float32),
                bk=np.ascontiguousarray(bk[hsl], dtype=np.float32),
                bv=np.ascontiguousarray(bv[hsl], dtype=np.float32),
                bo=(bo.astype(np.float32) if g == 0 else zeros_bo),
            )
        )
    return in_maps


def run_on_cores(in_maps, trace=False):
    global _compiled
    from concourse import bass_utils

    if _compiled is None:
        _compiled = _build_program()
    res = bass_utils.run_bass_kernel_spmd(
        _compiled, in_maps, core_ids=list(range(N_CORES)), trace=trace
    )
    return res


def kernel(q, k, v, mask, Wq, bq, Wk, bk, Wv, bv, Wo, bo):
    in_maps = _shard_inputs(q, k, v, mask, Wq, bq, Wk, bk, Wv, bv, Wo, bo)
    res = run_on_cores(in_maps)
    out = np.empty((B, S, D), dtype=np.float32)
    for b in range(B):
        partial = res.results[2 * b]["outT"] + res.results[2 * b + 1]["outT"]
        out[b] = partial.T
    return out
